# revision 6
# baseline (speedup 1.0000x reference)
"""Raw Bass Block kernel for DiagonalMatrixModel (out = x * diag broadcast).

Dataflow (per core, rows sharded 8-way: [1024, 4096] f32 in/out):
  - diag: one HWDGE DMA on the SP ring reads dg[4096] from HBM with a
    partition-stride-0 AP -> dtile [128, 4096] (every partition gets the
    full row).  No PE/PSUM broadcast chain.
  - x: 16 tiles of [128, 2048] (1 MiB each).  All loads stream on the
    ACT HWDGE ring; all stores stream on the SP HWDGE ring.  The SDMA
    engines round-robin between the two rings at packet granularity, so
    loads and stores mix from the moment the first multiply finishes and
    the fabric stays pegged at its ~435 GB/s combined ceiling.
  - DVE: in-place tensor_mul per tile (~2.7 us), always ahead of the
    ~4.6 us/tile steady-state DMA rate, so stores are never DVE-gated.
  - Single store-completion semaphore (order irrelevant, only the total
    count of 16*NT matters); per-tile load semaphores (FIFO per ring
    does not guarantee cross-engine inc ordering within one sem).
  - Bass-init head drains/memsets and block-end drains stripped
    post-build; completion is guaranteed by SP's wait on the store sem.
"""

import numpy as np

import concourse.bass as bass
import concourse.mybir as mybir
from concourse.bass_utils import run_bass_kernel_spmd

BATCH = 8192
SIZE = 4096
N_CORES = 8
ROWS = BATCH // N_CORES  # 1024
P = 128
NT = 16           # tiles: row-block i//2, col-block i%2
CB = SIZE // 2    # 2048 cols per tile

_CACHE: dict = {}


def _build() -> bass.Bass:
    nc = bass.Bass("TRN2", enable_asserts=False)
    f32 = mybir.dt.float32
    x = nc.dram_tensor("x", [ROWS, SIZE], f32, kind="ExternalInput")
    dg = nc.dram_tensor("diagonal", [SIZE], f32, kind="ExternalInput")
    out = nc.dram_tensor("out", [ROWS, SIZE], f32, kind="ExternalOutput")

    # One SBUF buffer per 128-row block; loads and multiplies work on
    # 1 MiB column halves, stores ship the full contiguous 2 MiB block.
    xrb = [nc.alloc_sbuf_tensor(f"xrb{r}", [P, SIZE], f32) for r in range(NT // 2)]
    dtile = nc.alloc_sbuf_tensor("dtile", [P, SIZE], f32)

    def rs(i):
        r = (i // 2) * P
        return slice(r, r + P)

    def cs(i):
        c = (i % 2) * CB
        return slice(c, c + CB)

    def half(i):
        # SBUF view of tile i: its row-block buffer, column half i%2.
        return xrb[i // 2].ap()[:, cs(i)]

    from contextlib import ExitStack

    with ExitStack() as es, nc.Block(no_gpsimd_drain=True) as block:
        sem_dg = es.enter_context(nc.semaphore("sem_dg"))
        sem_mul = es.enter_context(nc.semaphore("sem_mul"))
        sem_st = es.enter_context(nc.semaphore("sem_st"))
        sem_ld = [es.enter_context(nc.semaphore(f"sem_ld{i}")) for i in range(NT)]

        @block.scalar
        def _(act):
            # ACT HWDGE ring: upper half of the diag broadcast, then all
            # x loads issued back-to-back.
            act.dma_start(
                out=dtile.ap()[64:128, :],
                in_=dg[:].partition_broadcast(64),
            ).then_inc(sem_dg, 16)
            for i in range(NT):
                act.dma_start(out=half(i), in_=x[rs(i), cs(i)]).then_inc(
                    sem_ld[i], 16
                )

        @block.sync
        def _(sp):
            # SP HWDGE ring: lower half of the diag broadcast (also warms
            # the ring), then 8 full row-block stores (2 MiB contiguous)
            # as their two half-tile multiplies retire.
            sp.dma_start(
                out=dtile.ap()[0:64, :],
                in_=dg[:].partition_broadcast(64),
            ).then_inc(sem_dg, 16)
            for r in range(NT // 2):
                sp.wait_ge(sem_mul, 2 * r + 2)
                sp.dma_start(
                    out=out[r * P : (r + 1) * P, :], in_=xrb[r].ap()
                ).then_inc(sem_st, 16)
            sp.wait_ge(sem_st, 16 * (NT // 2))

        @block.vector
        def _(dve):
            dve.wait_ge(sem_dg, 32)
            for i in range(NT):
                dve.wait_ge(sem_ld[i], 16)
                dve.tensor_mul(
                    half(i), half(i), dtile.ap()[:, cs(i)]
                ).then_inc(sem_mul, 1)

    # Drop the Bass-init head drains/event-semaphores/const-memsets and the
    # block-end drains — completion is already guaranteed by SP's final wait
    # on the store-completion semaphore.
    blocks = nc.m.functions[0].blocks
    blocks[0].instructions = [
        inst
        for inst in blocks[0].instructions
        if type(inst).__name__ not in ("InstDrain", "InstEventSemaphore", "InstMemset")
    ]
    end_bb = blocks[-1]
    end_bb.instructions = [
        inst
        for inst in end_bb.instructions
        if type(inst).__name__ not in ("InstDrain", "InstEventSemaphore")
    ]
    return nc


def kernel(x: np.ndarray, diagonal: np.ndarray) -> np.ndarray:
    if "nc" not in _CACHE:
        _CACHE["nc"] = _build()
    nc = _CACHE["nc"]

    x = np.ascontiguousarray(np.asarray(x, dtype=np.float32))
    diagonal = np.ascontiguousarray(np.asarray(diagonal, dtype=np.float32))

    shards = np.split(x, N_CORES, axis=0)
    in_maps = [{"x": s, "diagonal": diagonal} for s in shards]
    res = run_bass_kernel_spmd(nc, in_maps, list(range(N_CORES))).results
    return np.concatenate([r["out"] for r in res], axis=0)


# revision 9
# speedup vs baseline: 1.0738x; 1.0738x over previous
"""Raw Bass Block kernel for DiagonalMatrixModel (out = x * diag broadcast).

Dataflow (per core, rows sharded 8-way: [1024, 4096] f32 in/out):
  - diag: two HWDGE DMAs on the SP ring read dg halves from HBM with a
    partition-stride-0 AP -> dtile [128, 4096] (every partition gets the
    full row).  Split in two so the first multiply only waits for the
    first 1 MiB half.  No PE/PSUM broadcast chain.
  - x: 16 tiles of [128, 2048] (1 MiB each).  All loads stream on the
    ACT HWDGE ring; stores stream on the SP HWDGE ring.  Equal transfer
    shapes on both rings matter: the SDMA engines round-robin between
    rings at *packet* granularity, so equal descriptor sizes give a fair
    byte split and the fabric stays pegged at its ~435 GB/s combined
    ceiling.  The last two stores ride the ACT ring instead (queued
    behind the loads, which have drained by then) so the store-only tail
    drains on both rings at once.
  - DVE: in-place tensor_mul per tile (~2.75 us), gated per diag half.
  - Single store-completion semaphore (only the total of 16*NT matters);
    per-tile load semaphores (cross-engine inc ordering within one sem
    is not guaranteed).
  - Bass-init head drains/memsets and block-end drains stripped
    post-build; completion is guaranteed by the final waits on the
    store-completion semaphore.
"""

import numpy as np

import concourse.bass as bass
import concourse.mybir as mybir
from concourse.bass_utils import run_bass_kernel_spmd

BATCH = 8192
SIZE = 4096
N_CORES = 8
ROWS = BATCH // N_CORES  # 1024
P = 128
NT = 16           # tiles: row-block i//2, col-block i%2
CB = SIZE // 2    # 2048
N_ACT_ST = 2      # stores routed to the ACT ring (tail drain on 2 rings)

_CACHE: dict = {}


def _build() -> bass.Bass:
    nc = bass.Bass("TRN2", enable_asserts=False)
    f32 = mybir.dt.float32
    x = nc.dram_tensor("x", [ROWS, SIZE], f32, kind="ExternalInput")
    dg = nc.dram_tensor("diagonal", [SIZE], f32, kind="ExternalInput")
    out = nc.dram_tensor("out", [ROWS, SIZE], f32, kind="ExternalOutput")

    xt = [nc.alloc_sbuf_tensor(f"xt{i}", [P, CB], f32) for i in range(NT)]
    dtile = nc.alloc_sbuf_tensor("dtile", [P, SIZE], f32)

    def rs(i):
        r = (i // 2) * P
        return slice(r, r + P)

    def cs(i):
        c = (i % 2) * CB
        return slice(c, c + CB)

    from contextlib import ExitStack

    with ExitStack() as es, nc.Block(no_gpsimd_drain=True) as block:
        sem_dg = [es.enter_context(nc.semaphore(f"sem_dg{h}")) for h in range(2)]
        sem_mul = es.enter_context(nc.semaphore("sem_mul"))
        sem_st = es.enter_context(nc.semaphore("sem_st"))
        sem_ld = [es.enter_context(nc.semaphore(f"sem_ld{i}")) for i in range(NT)]

        def store(eng, i):
            eng.wait_ge(sem_mul, i + 1)
            eng.dma_start(out=out[rs(i), cs(i)], in_=xt[i].ap()).then_inc(
                sem_st, 16
            )

        @block.scalar
        def _(act):
            # ACT HWDGE ring: all x loads back-to-back, then the last two
            # stores (they queue behind the loads and drain in the tail).
            for i in range(NT):
                act.dma_start(out=xt[i].ap(), in_=x[rs(i), cs(i)]).then_inc(
                    sem_ld[i], 16
                )
            for i in range(NT - N_ACT_ST, NT):
                store(act, i)

        @block.sync
        def _(sp):
            # SP HWDGE ring: diag halves first (also warms the ring), then
            # the stores as their multiplies retire.
            for h in range(2):
                sp.dma_start(
                    out=dtile.ap()[:, h * CB : (h + 1) * CB],
                    in_=dg[h * CB : (h + 1) * CB].partition_broadcast(P),
                ).then_inc(sem_dg[h], 16)
            for i in range(NT - N_ACT_ST):
                store(sp, i)
            sp.wait_ge(sem_st, 16 * NT)

        @block.vector
        def _(dve):
            for i in range(NT):
                if i < 2:
                    dve.wait_ge(sem_dg[i % 2], 16)
                dve.wait_ge(sem_ld[i], 16)
                dve.tensor_mul(
                    xt[i].ap(), xt[i].ap(), dtile.ap()[:, cs(i)]
                ).then_inc(sem_mul, 1)

    # Drop the Bass-init head drains/event-semaphores/const-memsets and the
    # block-end drains — completion is already guaranteed by the final waits
    # on the store-completion semaphore.
    blocks = nc.m.functions[0].blocks
    blocks[0].instructions = [
        inst
        for inst in blocks[0].instructions
        if type(inst).__name__ not in ("InstDrain", "InstEventSemaphore", "InstMemset")
    ]
    end_bb = blocks[-1]
    end_bb.instructions = [
        inst
        for inst in end_bb.instructions
        if type(inst).__name__ not in ("InstDrain", "InstEventSemaphore")
    ]
    return nc


def kernel(x: np.ndarray, diagonal: np.ndarray) -> np.ndarray:
    if "nc" not in _CACHE:
        _CACHE["nc"] = _build()
    nc = _CACHE["nc"]

    x = np.ascontiguousarray(np.asarray(x, dtype=np.float32))
    diagonal = np.ascontiguousarray(np.asarray(diagonal, dtype=np.float32))

    shards = np.split(x, N_CORES, axis=0)
    in_maps = [{"x": s, "diagonal": diagonal} for s in shards]
    res = run_bass_kernel_spmd(nc, in_maps, list(range(N_CORES))).results
    return np.concatenate([r["out"] for r in res], axis=0)


# revision 11
# speedup vs baseline: 1.2391x; 1.1539x over previous
"""Raw Bass Block kernel for DiagonalMatrixModel (out = x * diag broadcast).

Dataflow (per core, rows sharded 8-way: [1024, 4096] f32 in/out):
  - diag: two HWDGE DMAs on the SP ring read dg halves from HBM with a
    partition-stride-0 AP -> dtile [128, 4096] (every partition gets the
    full row).  Split in two so the first multiply only waits for the
    first 1 MiB half.  No PE/PSUM broadcast chain.
  - x: 16 tiles of [128, 2048] (1 MiB each).  All loads stream on the
    ACT HWDGE ring; stores stream on the SP HWDGE ring.  Equal transfer
    shapes on both rings matter: the SDMA engines round-robin between
    rings at *packet* granularity, so equal descriptor sizes give a fair
    byte split and the fabric stays pegged at its ~435 GB/s combined
    ceiling.  The last two stores ride the ACT ring instead (queued
    behind the loads, which have drained by then) so the store-only tail
    drains on both rings at once.
  - DVE: in-place tensor_mul per tile (~2.75 us), gated per diag half.
  - Single store-completion semaphore (only the total of 16*NT matters);
    per-tile load semaphores (cross-engine inc ordering within one sem
    is not guaranteed).
  - Bass-init head drains/memsets and block-end drains stripped
    post-build; completion is guaranteed by the final waits on the
    store-completion semaphore.
"""

import numpy as np

import concourse.bass as bass
import concourse.mybir as mybir
from concourse.bass_utils import run_bass_kernel_spmd

BATCH = 8192
SIZE = 4096
N_CORES = 8
ROWS = BATCH // N_CORES  # 1024
P = 128
NT = 16           # tiles: row-block i//2, col-block i%2
CB = SIZE // 2    # 2048
N_ACT_ST = 2      # stores routed to the ACT ring (tail drain on 2 rings)

_CACHE: dict = {}


def _build() -> bass.Bass:
    nc = bass.Bass("TRN2", enable_asserts=False)
    f32 = mybir.dt.float32
    x = nc.dram_tensor("x", [ROWS, SIZE], f32, kind="ExternalInput")
    dg = nc.dram_tensor("diagonal", [SIZE], f32, kind="ExternalInput")
    out = nc.dram_tensor("out", [ROWS, SIZE], f32, kind="ExternalOutput")

    xt = [nc.alloc_sbuf_tensor(f"xt{i}", [P, CB], f32) for i in range(NT)]
    dtile = nc.alloc_sbuf_tensor("dtile", [P, SIZE], f32)

    def rs(i):
        r = (i // 2) * P
        return slice(r, r + P)

    def cs(i):
        c = (i % 2) * CB
        return slice(c, c + CB)

    from contextlib import ExitStack

    with ExitStack() as es, nc.Block(no_gpsimd_drain=True) as block:
        sem_dg = [es.enter_context(nc.semaphore(f"sem_dg{h}")) for h in range(2)]
        sem_mul = es.enter_context(nc.semaphore("sem_mul"))
        sem_st = es.enter_context(nc.semaphore("sem_st"))
        sem_ld = [es.enter_context(nc.semaphore(f"sem_ld{i}")) for i in range(NT)]

        def store(eng, i):
            eng.wait_ge(sem_mul, i + 1)
            eng.dma_start(out=out[rs(i), cs(i)], in_=xt[i].ap()).then_inc(
                sem_st, 16
            )

        @block.scalar
        def _(act):
            # ACT HWDGE ring: all x loads back-to-back, then the last two
            # stores (they queue behind the loads and drain in the tail).
            for i in range(NT):
                act.dma_start(out=xt[i].ap(), in_=x[rs(i), cs(i)]).then_inc(
                    sem_ld[i], 16
                )
            for i in range(NT - N_ACT_ST, NT):
                store(act, i)

        @block.sync
        def _(sp):
            # SP HWDGE ring: diag halves first (also warms the ring), then
            # the stores as their multiplies retire.
            for h in range(2):
                sp.dma_start(
                    out=dtile.ap()[:, h * CB : (h + 1) * CB],
                    in_=dg[h * CB : (h + 1) * CB].partition_broadcast(P),
                ).then_inc(sem_dg[h], 16)
            for i in range(NT - N_ACT_ST):
                store(sp, i)
            sp.wait_ge(sem_st, 16 * NT)

        @block.vector
        def _(dve):
            for i in range(NT):
                if i < 2:
                    dve.wait_ge(sem_dg[i % 2], 16)
                dve.wait_ge(sem_ld[i], 16)
                dve.tensor_mul(
                    xt[i].ap(), xt[i].ap(), dtile.ap()[:, cs(i)]
                ).then_inc(sem_mul, 1)

    # Drop the Bass-init head drains/event-semaphores/const-memsets and the
    # block-end drains — completion is already guaranteed by the final waits
    # on the store-completion semaphore.
    blocks = nc.m.functions[0].blocks
    blocks[0].instructions = [
        inst
        for inst in blocks[0].instructions
        if type(inst).__name__ not in ("InstDrain", "InstEventSemaphore", "InstMemset")
    ]
    end_bb = blocks[-1]
    end_bb.instructions = [
        inst
        for inst in end_bb.instructions
        if type(inst).__name__ not in ("InstDrain", "InstEventSemaphore")
    ]
    return nc


def kernel(x: np.ndarray, diagonal: np.ndarray) -> np.ndarray:
    if "nc" not in _CACHE:
        _CACHE["nc"] = _build()
    nc = _CACHE["nc"]

    x = np.ascontiguousarray(np.asarray(x, dtype=np.float32))
    diagonal = np.ascontiguousarray(np.asarray(diagonal, dtype=np.float32))

    shards = np.split(x, N_CORES, axis=0)
    in_maps = [{"x": s, "diagonal": diagonal} for s in shards]
    res = run_bass_kernel_spmd(nc, in_maps, list(range(N_CORES))).results
    return np.concatenate([r["out"] for r in res], axis=0)
